# revision 1
# baseline (speedup 1.0000x reference)
"""BrushStroke splat kernel for 8 trn2 NeuronCores.

out[b,c,y,x] = mean_n sum_{p,q} Fy[b,n,y,p] Fx[b,n,x,q] patches[b,n,c,p,q]
with Fx/Fy separable Gaussian filter banks (sigma=0.1) normalized over a
padded spatial axis.

Strategy (per core, 2 batches of 64 strokes):
 - The Gaussian is Toeplitz in (x - q): build one row E[n, t] =
   exp(-(t - center_n)^2 / (2 sigma^2)) of length 319 per stroke
   (ScalarE Square+Exp, strokes on partitions), then DMA-gather shifted
   windows into per-group filter tiles [(j,q'), x] (q' = reversed q).
 - MM1 per (group of 4 strokes, channel): one full-array f32r matmul
   with a block-diagonal lhsT [128,128] holding the 4 strokes' patch
   blocks scaled by the Fx normalizers -> t[(j,p'), x] in PSUM.
 - MM2 per (ytile, channel): 16 chained f32r matmuls accumulate
   sum_g FyN_g^T @ t_g into PSUM [y, x]; drain with x1/64.
Batch-parallel across cores; no collectives.
"""
import sys, types
import numpy as np

IMAGE = 256
PAD = 16
EPS = 1e-7
SIGMA2 = 2.0 * 0.1 ** 2
B, N, C, PH, PW = 16, 64, 3, 32, 32
NCORES = 8
BLOC = B // NCORES          # 2 batches per core
NG = N // 4                 # 16 groups of 4 strokes
ET = IMAGE + 2 * PAD + PW - 1   # 319: E row length


def _install_patches():
    if 'antenv.axon_hooks' not in sys.modules:
        mod = types.ModuleType('antenv.axon_hooks')
        mod._hook = None
        mod.set_axon_ntff_profile_hook = lambda h: setattr(mod, '_hook', h)
        mod.get_axon_ntff_profile_hook = lambda: mod._hook
        sys.modules['antenv.axon_hooks'] = mod
        try:
            from trn_agent_boot.trn_boot import _ntff_profile_via_ctypes
            hook = _ntff_profile_via_ctypes('/opt/axon/libaxon_pjrt.so')
            if hook is not None:
                mod.set_axon_ntff_profile_hook(hook)
        except Exception:
            pass

    import concourse.tile as tile
    import concourse.bass_utils as bass_utils
    from concourse.vector_clock import ScopedClock

    bass_utils.upload_artifacts = lambda tmpdir: 'local://' + tmpdir

    if getattr(tile.TileContext._drain_and_barrier, '_patched', False):
        return

    def _drain_and_barrier(self, tick_clock, wait_clock):
        nc = self.nc
        drain_inst = nc.sync.drain()
        wait_clock.add_sem_waits(
            drain_inst.ins, ScopedClock({None: tick_clock.global_clock}))
        si = drain_inst.ins.sync_info
        waits = list(si.on_wait or [])
        si.on_wait = []
        for w in waits:
            nop = nc.sync.nop()
            nop.ins.sync_info = type(si)(on_wait=[w], on_update=[])
        nc.all_engine_barrier()
        popped = nc._tile_sem_poison_stack.pop()
        assert popped is self._sem_poison
        nc.clear_and_free_semaphores(list(self.sems.allocated().values()))
        nc.all_engine_barrier()

    _drain_and_barrier._patched = True
    tile.TileContext._drain_and_barrier = _drain_and_barrier


def _split_multi_waits(nc):
    """This walrus accepts at most one sync wait per instruction; hoist
    extras onto same-engine NoOps inserted just before."""
    import bass_rust
    n_new = [0]

    def fresh_nop(engine, wait, si_type):
        n_new[0] += 1
        nop = bass_rust.InstNoOp(name=f'I-waitsplit-{n_new[0]}', ins=[], outs=[])
        nop.engine = engine
        nop.sync_info = si_type(on_wait=[wait], on_update=[])
        return nop

    for fn in nc.m.functions:
        for blk in fn.blocks:
            insts = blk.instructions
            i = 0
            while i < len(insts):
                inst = insts[i]
                si = inst.sync_info
                if si is not None and si.on_wait and len(si.on_wait) > 1:
                    waits = list(si.on_wait)
                    si.on_wait = [waits[-1]]
                    for k, w in enumerate(waits[:-1]):
                        insts.insert(i + k, fresh_nop(inst.engine, w, type(si)))
                    i += len(waits) - 1
                i += 1


_PROGRAM = None


def _build_program():
    global _PROGRAM
    if _PROGRAM is not None:
        return _PROGRAM
    _install_patches()
    import concourse.bass as bass
    import concourse.tile as tile
    from concourse import mybir
    from bass_rust import AP

    f32 = mybir.dt.float32
    f32r = mybir.dt.float32r
    AF = mybir.ActivationFunctionType
    AX = mybir.AxisListType

    nc = bass.Bass('TRN2', target_bir_lowering=False, debug=False,
                   num_devices=NCORES)
    # inputs (per core): brush coords by (batch,coord) rows; patches in
    # reversed-(p,q) block layout; 4x4 identity for the tiny transpose
    g_in = nc.declare_dram_parameter('g_in', [4, N], f32, isOutput=False)
    pt_in = nc.declare_dram_parameter('pt_in', [BLOC, 128, NG * C * PH], f32,
                                      isOutput=False)
    id4 = nc.declare_dram_parameter('id4', [4, 4], f32, isOutput=False)
    y_out = nc.declare_dram_parameter('y_out', [BLOC, C, IMAGE, IMAGE], f32,
                                      isOutput=True)

    with tile.TileContext(nc) as tc:
        with tc.tile_pool(name='glob', bufs=1) as gp, \
             tc.tile_pool(name='ps_init', bufs=1, space='PSUM') as psi:
            # ---- brush normalization -> centers bias vectors ----
            bc = gp.tile([4, N], f32)
            nc.sync.dma_start(bc[:], g_in[:])
            idt = gp.tile([4, 4], f32)
            nc.sync.dma_start(idt[:], id4[:])

            mn = gp.tile([4, 1], f32)
            mx = gp.tile([4, 1], f32)
            nc.vector.tensor_reduce(mn[:], bc[:], axis=AX.X,
                                    op=mybir.AluOpType.min)
            nc.vector.reduce_max(mx[:], bc[:], axis=AX.X)
            rng = gp.tile([4, 1], f32)
            nc.vector.tensor_sub(rng[:], mx[:], mn[:])
            nc.vector.tensor_scalar_add(rng[:], rng[:], EPS)
            inv = gp.tile([4, 1], f32)
            nc.vector.reciprocal(inv[:], rng[:])
            nc.vector.tensor_scalar_mul(inv[:], inv[:], float(IMAGE))
            gn = gp.tile([4, N], f32)
            nc.vector.tensor_scalar_sub(gn[:], bc[:], mn[:])
            nc.vector.tensor_scalar_mul(gn[:], gn[:], inv[:])

            # transpose [4,N] -> [N,4]; replicate to both 64-row halves
            tp_ps = psi.tile([N, 4], f32)
            nc.tensor.transpose(tp_ps[:], gn[:], idt[:])
            tp = gp.tile([128, 4], f32)
            nc.scalar.copy(tp[0:N, :], tp_ps[:])
            nc.vector.tensor_copy(tp[N:128, :], tp_ps[:])

            # bias = -(g + 31.5 or 31.6); rows 0:64 from cols 0/1 (batch0),
            # rows 64:128 from cols 2/3 (batch1)
            bias_x = gp.tile([128, 1], f32)
            bias_y = gp.tile([128, 1], f32)
            CX = PW / 2 - 0.5 + PAD      # 31.5
            CY = PW / 2 - 0.4 + PAD      # 31.6
            MUL, SUB = mybir.AluOpType.mult, mybir.AluOpType.subtract
            nc.vector.tensor_scalar(bias_x[0:N, :], tp[0:N, 0:1],
                                    -1.0, CX, MUL, SUB)
            nc.vector.tensor_scalar(bias_x[N:128, :], tp[N:128, 2:3],
                                    -1.0, CX, MUL, SUB)
            nc.vector.tensor_scalar(bias_y[0:N, :], tp[0:N, 1:2],
                                    -1.0, CY, MUL, SUB)
            nc.vector.tensor_scalar(bias_y[N:128, :], tp[N:128, 3:4],
                                    -1.0, CY, MUL, SUB)

            # ---- E rows: exp(-(t - c)^2 / SIGMA2), strokes on partitions --
            it = gp.tile([128, ET], f32)
            nc.gpsimd.iota(it[:], pattern=[[1, ET]], base=0,
                           channel_multiplier=0,
                           allow_small_or_imprecise_dtypes=True)
            sq = gp.tile([128, ET], f32)
            E_x = gp.tile([128, ET], f32r)
            E_y = gp.tile([128, ET], f32r)
            nc.scalar.activation(sq[:], it[:], AF.Square, bias=bias_x[:],
                                 scale=1.0)
            nc.scalar.activation(E_x[:], sq[:], AF.Exp, bias=0.0,
                                 scale=-1.0 / SIGMA2)
            sq2 = gp.tile([128, ET], f32)
            nc.scalar.activation(sq2[:], it[:], AF.Square, bias=bias_y[:],
                                 scale=1.0)
            nc.scalar.activation(E_y[:], sq2[:], AF.Exp, bias=0.0,
                                 scale=-1.0 / SIGMA2)

            # bounce E through DRAM: DRAM-source gathers spread across all
            # 16 DMA engines (SBUF->SBUF is pinned to 4)
            E_x_dram = nc.dram_tensor('E_x_dram', [128, ET], f32r)
            E_y_dram = nc.dram_tensor('E_y_dram', [128, ET], f32r)
            nc.sync.dma_start(E_x_dram[:], E_x[:])
            nc.sync.dma_start(E_y_dram[:], E_y[:])

            # ---- per-batch main loop ----
            for b in range(BLOC):
                with tc.tile_pool(name=f'b{b}', bufs=1) as bp:
                    # compact patches preload: [(j,q'), (g,c,p')]
                    ptc = bp.tile([128, NG * C * PH], f32, name=f'ptc{b}',
                                  tag=f'ptc{b}')
                    nc.sync.dma_start(ptc[:], pt_in[b])
                    # whole-batch block-diagonal lhsT: zeroed once, then 4
                    # strided mega-copies (one per stroke slot j) place every
                    # group's patch blocks; copy is also the f32r rounding
                    ps_all = bp.tile([128, 128 * C * NG], f32r,
                                     name=f'psall{b}', tag=f'psall{b}')
                    nc.gpsimd.memset(ps_all.bitcast(f32)[:], 0.0)
                    for j in range(4):
                        dst0 = ps_all[32 * j:32 * j + 1, 32 * j:32 * j + 1]
                        dst = AP(ps_all.tensor, dst0.offset,
                                 [[128 * C * NG, 32], [128 * C, NG],
                                  [128, C], [1, PH]])
                        src0 = ptc[32 * j:32 * j + 1, 0:1]
                        srcap = AP(ptc.tensor, src0.offset,
                                   [[NG * C * PH, 32], [C * PH, NG],
                                    [PH, C], [1, PH]])
                        nc.vector.tensor_copy(dst, srcap)
                    t_tiles = {}
                    fyn_tiles = {}
                    psa_cm = tc.tile_pool(name=f'ps_a{b}', bufs=2,
                                          space='PSUM')
                    psa = psa_cm.__enter__()
                    GW = ET - PW + 1      # 288 gathered cols per filter
                    for g in range(NG):
                        pp = g % 2
                        # gathers: dest[(j,q'), x] = E[64b+4g+j, q' + x]
                        dmae = [nc.sync, nc.scalar][g % 2]
                        dmae2 = [nc.scalar, nc.sync][g % 2]
                        fx_g = bp.tile([128, GW], f32r,
                                       name=f'fxg{b}{pp}', tag=f'fxg{b}{pp}')
                        src = AP(E_x_dram, (N * b + 4 * g) * ET,
                                 [[ET, 4], [1, PW], [1, GW]])
                        dmae.dma_start(fx_g[:], src)
                        fy_g = bp.tile([128, GW], f32r,
                                       name=f'fyg{b}{pp}', tag=f'fyg{b}{pp}')
                        src = AP(E_y_dram, (N * b + 4 * g) * ET,
                                 [[ET, 4], [1, PW], [1, GW]])
                        dmae2.dma_start(fy_g[:], src)

                        # window sums + reciprocal normalizers
                        wsx = bp.tile([128, 1], f32, name=f'wsx{b}{pp}', tag=f'wsx{b}{pp}')
                        scr = bp.tile([128, GW], f32r,
                                      name=f'scr{b}{pp}', tag=f'scr{b}{pp}')
                        nc.scalar.activation(scr.bitcast(f32)[:],
                                             fx_g.bitcast(f32)[:],
                                             AF.Copy, scale=1.0,
                                             accum_out=wsx[:])
                        nc.vector.tensor_scalar_add(wsx[:], wsx[:], EPS)
                        invx = bp.tile([128, 1], f32, name=f'ivx{b}{pp}', tag=f'ivx{b}{pp}')
                        nc.vector.reciprocal(invx[:], wsx[:])

                        wsy = bp.tile([128, 1], f32, name=f'wsy{b}{pp}', tag=f'wsy{b}{pp}')
                        nc.vector.reduce_sum(wsy[:], fy_g.bitcast(f32)[:],
                                             axis=AX.X)
                        nc.vector.tensor_scalar_add(wsy[:], wsy[:], EPS)
                        invy = bp.tile([128, 1], f32, name=f'ivy{b}{pp}', tag=f'ivy{b}{pp}')
                        nc.vector.reciprocal(invy[:], wsy[:])

                        # normalized Fy [128, 256] f32r (y = 0..255)
                        fyn = bp.tile([128, IMAGE], f32r, name=f'fyn{b}_{g}', tag=f'fyn{b}_{g}')
                        nc.vector.tensor_scalar_mul(
                            fyn[:], fy_g.bitcast(f32)[:, PAD:PAD + IMAGE],
                            invy[:])
                        fyn_tiles[g] = fyn

                        # normalized Fx (f32r rounding + invx fold)
                        fxn = bp.tile([128, IMAGE], f32r, name='fxn',
                                      tag=f'fxn{b}', bufs=2)
                        nc.vector.tensor_scalar_mul(
                            fxn[:], fx_g.bitcast(f32)[:, PAD:PAD + IMAGE],
                            invx[:])

                        # MM1: one full-array f32r matmul per channel
                        for c in range(C):
                            p1 = psa.tile([128, IMAGE], f32, name=f'p1_{c}',
                                          tag=f'p1_{c}')
                            nc.tensor.matmul(
                                p1[:],
                                ps_all[:, 384 * g + 128 * c:
                                       384 * g + 128 * c + 128],
                                fxn[:], start=True, stop=True)
                            tt = bp.tile([128, IMAGE], f32r,
                                         name=f't{b}_{g}_{c}', tag=f't{b}_{g}_{c}')
                            if c == 1:
                                nc.vector.tensor_copy(tt[:], p1[:])
                            else:
                                nc.scalar.copy(tt[:], p1[:])
                            t_tiles[(g, c)] = tt

                    psa_cm.__exit__(None, None, None)
                    # ---- MM2: accumulate over groups ----
                    with tc.tile_pool(name=f'ps_b{b}', bufs=2,
                                      space='PSUM') as psb:
                        for yt in range(2):
                            for c in range(C):
                                acc = psb.tile([128, IMAGE], f32,
                                               name=f'acc{c}', tag=f'acc{c}')
                                for g in range(NG):
                                    nc.tensor.matmul(
                                        acc[:],
                                        fyn_tiles[g][:, 128 * yt:128 * yt + 128],
                                        t_tiles[(g, c)][:],
                                        start=(g == 0), stop=(g == NG - 1))
                                ob = bp.tile([128, IMAGE], f32,
                                             name=f'ob{c}', tag=f'ob{c}')
                                nc.scalar.mul(ob[:], acc[:], 1.0 / N)
                                nc.sync.dma_start(
                                    y_out[b, c, 128 * yt:128 * yt + 128, :],
                                    ob[:])

    _split_multi_waits(nc)
    _PROGRAM = nc
    return nc


def _make_in_maps(brushes: np.ndarray, patches: np.ndarray):
    brushes = np.asarray(brushes, dtype=np.float32)
    patches = np.asarray(patches, dtype=np.float32)
    id4 = np.eye(4, dtype=np.float32)
    in_maps = []
    for k in range(NCORES):
        bsl = brushes[BLOC * k: BLOC * (k + 1)]        # [2, 64, 2]
        g_in = np.ascontiguousarray(
            bsl.transpose(0, 2, 1).reshape(4, N))       # rows b0x,b0y,b1x,b1y
        psl = patches[BLOC * k: BLOC * (k + 1)]         # [2, 64, 3, 32, 32]
        pr = psl.reshape(BLOC, NG, 4, C, PH, PW)[..., ::-1, ::-1]
        # -> [b, j, q', g, c, p'] -> [b, 128, NG*C*PH]
        pt = np.ascontiguousarray(pr.transpose(0, 2, 5, 1, 3, 4)).reshape(
            BLOC, 128, NG * C * PH)
        in_maps.append({'g_in': g_in, 'pt_in': pt, 'id4': id4})
    return in_maps


def kernel(brushes: np.ndarray, patches: np.ndarray) -> np.ndarray:
    from concourse.bass_utils import run_bass_kernel_spmd

    nc = _build_program()
    in_maps = _make_in_maps(brushes, patches)
    res = run_bass_kernel_spmd(nc, in_maps, list(range(NCORES)))
    out = np.concatenate([res.results[k]['y_out'] for k in range(NCORES)],
                         axis=0)
    return out



# revision 3
# speedup vs baseline: 1.6436x; 1.6436x over previous
"""BrushStroke splat kernel for 8 trn2 NeuronCores (v2).

out[b,c,y,x] = mean_n sum_{p,q} Fy[b,n,y,p] Fx[b,n,x,q] patches[b,n,c,p,q]
with Fx/Fy separable Gaussian filter banks (sigma=0.1) normalized over a
padded spatial axis.

v2 strategy (per core, 2 batches of 64 strokes, batch-parallel across
cores, no collectives):
 - E rows E[r, t] = exp(-(t - c_r)^2 / (2 sigma^2)) of length 319, one
   per (stroke, batch) on partition r = 32j + 2g + b (j = stroke slot,
   g = group, b = batch).
 - All 64x32 filter normalizers are computed in a one-time prologue:
   window sum D[r,q'] = T - prefix - suffix via tensor_tensor_scan,
   one reciprocal, then a DVE 32x32 block transpose yields per-partition
   scale tables IVX/IVY [(j,q'), (g,b)].
 - Per group: DMA-gather shifted E windows [128,256]; Fx normalizer is
   one tensor_scalar_mul; Fy windows feed MM2 raw (normalizer folded
   into the MM1 PSUM drain).
 - MM1 per (g,c): one full-array f32r matmul with a block-diagonal lhsT
   DMA'd straight from DRAM (zeros embedded host-side).
 - MM2 per (b,yt): 16 chained matmuls x {512-wide c0c1, 256-wide c2}.
 - A few warmup matmuls during the prologue keep the PE HAM warm.
"""
import sys, types
import numpy as np

IMAGE = 256
PAD = 16
EPS = 1e-7
SIGMA2 = 2.0 * 0.1 ** 2
B, N, C, PH, PW = 16, 64, 3, 32, 32
NCORES = 8
BLOC = B // NCORES          # 2 batches per core
NG = N // 4                 # 16 groups of 4 strokes
ET = IMAGE + 2 * PAD + PW - 1   # 319: E row length
WLEN = IMAGE + 2 * PAD          # 288: padded-axis (normalizer window) length


def _install_patches():
    if 'antenv.axon_hooks' not in sys.modules:
        mod = types.ModuleType('antenv.axon_hooks')
        mod._hook = None
        mod.set_axon_ntff_profile_hook = lambda h: setattr(mod, '_hook', h)
        mod.get_axon_ntff_profile_hook = lambda: mod._hook
        sys.modules['antenv.axon_hooks'] = mod
        try:
            from trn_agent_boot.trn_boot import _ntff_profile_via_ctypes
            hook = _ntff_profile_via_ctypes('/opt/axon/libaxon_pjrt.so')
            if hook is not None:
                mod.set_axon_ntff_profile_hook(hook)
        except Exception:
            pass

    import concourse.tile as tile
    import concourse.bass_utils as bass_utils
    from concourse.vector_clock import ScopedClock

    bass_utils.upload_artifacts = lambda tmpdir: 'local://' + tmpdir

    if getattr(tile.TileContext._drain_and_barrier, '_patched', False):
        return

    def _drain_and_barrier(self, tick_clock, wait_clock):
        nc = self.nc
        drain_inst = nc.sync.drain()
        wait_clock.add_sem_waits(
            drain_inst.ins, ScopedClock({None: tick_clock.global_clock}))
        si = drain_inst.ins.sync_info
        waits = list(si.on_wait or [])
        si.on_wait = []
        for w in waits:
            nop = nc.sync.nop()
            nop.ins.sync_info = type(si)(on_wait=[w], on_update=[])
        nc.all_engine_barrier()
        popped = nc._tile_sem_poison_stack.pop()
        assert popped is self._sem_poison
        nc.clear_and_free_semaphores(list(self.sems.allocated().values()))
        nc.all_engine_barrier()

    _drain_and_barrier._patched = True
    tile.TileContext._drain_and_barrier = _drain_and_barrier


def _split_multi_waits(nc):
    """This walrus accepts at most one sync wait per instruction; hoist
    extras onto same-engine NoOps inserted just before."""
    import bass_rust
    n_new = [0]

    def fresh_nop(engine, wait, si_type):
        n_new[0] += 1
        nop = bass_rust.InstNoOp(name=f'I-waitsplit-{n_new[0]}', ins=[], outs=[])
        nop.engine = engine
        nop.sync_info = si_type(on_wait=[wait], on_update=[])
        return nop

    for fn in nc.m.functions:
        for blk in fn.blocks:
            insts = blk.instructions
            i = 0
            while i < len(insts):
                inst = insts[i]
                si = inst.sync_info
                if si is not None and si.on_wait and len(si.on_wait) > 1:
                    waits = list(si.on_wait)
                    si.on_wait = [waits[-1]]
                    for k, w in enumerate(waits[:-1]):
                        insts.insert(i + k, fresh_nop(inst.engine, w, type(si)))
                    i += len(waits) - 1
                i += 1


_PROGRAM = None


def _build_program():
    global _PROGRAM
    if _PROGRAM is not None:
        return _PROGRAM
    _install_patches()
    import concourse.bass as bass
    import concourse.tile as tile
    from concourse import mybir
    from bass_rust import AP

    f32 = mybir.dt.float32
    f32r = mybir.dt.float32r
    AF = mybir.ActivationFunctionType
    AX = mybir.AxisListType
    ALU = mybir.AluOpType

    nc = bass.Bass('TRN2', target_bir_lowering=False, debug=False,
                   num_devices=NCORES)
    # per-core inputs:
    #  g2   [4,128]: rows (x_b0, x_b1, y_b0, y_b1); col r = stroke nu(r)
    #  bsel [128,1]: r % 2 (batch-select mask for bias build)
    #  pt2  [2,128,6144]: full block-diagonal lhsT content (zeros embedded)
    g2_in = nc.declare_dram_parameter('g2', [4, 128], f32, isOutput=False)
    bsel_in = nc.declare_dram_parameter('bsel', [128, 1], f32, isOutput=False)
    pt_in = nc.declare_dram_parameter('pt2', [BLOC, 128, 128 * C * NG], f32,
                                      isOutput=False)
    id4_in = nc.declare_dram_parameter('id4', [4, 4], f32, isOutput=False)
    y_out = nc.declare_dram_parameter('y_out', [BLOC, C, IMAGE, IMAGE], f32,
                                      isOutput=True)

    E_x_dram = nc.dram_tensor('E_x_dram', [128, ET], f32)
    E_y_dram = nc.dram_tensor('E_y_dram', [128, ET], f32)

    with tile.TileContext(nc) as tc:
        with tc.tile_pool(name='glob', bufs=1) as gp, \
             tc.tile_pool(name='fyp', bufs=2) as fyp, \
             tc.tile_pool(name='fxp', bufs=4) as fxp, \
             tc.tile_pool(name='tgp', bufs=1) as tgp, \
             tc.tile_pool(name='obp', bufs=2) as obp, \
             tc.tile_pool(name='mm1ps', bufs=1, space='PSUM') as mm1ps, \
             tc.tile_pool(name='mm2ps', bufs=2, space='PSUM') as mm2ps:
            # ---- input DMAs ----
            g2 = gp.tile([4, 128], f32)
            nc.sync.dma_start(g2[:], g2_in[:])
            bsel = gp.tile([128, 1], f32)
            nc.sync.dma_start(bsel[:], bsel_in[:])
            idt = gp.tile([4, 4], f32)
            nc.sync.dma_start(idt[:], id4_in[:])

            # block-diagonal lhsT tiles, filled straight from DRAM
            psall = []
            for b in range(BLOC):
                ps = gp.tile([128, 128 * C * NG], f32r, name=f'psall{b}')
                for j in range(4):
                    nc.gpsimd.dma_start(ps[32 * j:32 * j + 32, :],
                                        pt_in[b, 32 * j:32 * j + 32, :])
                psall.append(ps)

            # ---- brush normalization -> bias vectors ----
            mn = gp.tile([4, 1], f32)
            mx = gp.tile([4, 1], f32)
            nc.vector.tensor_reduce(mn[:], g2[:], axis=AX.X, op=ALU.min)
            nc.vector.reduce_max(mx[:], g2[:], axis=AX.X)
            rng = gp.tile([4, 1], f32)
            nc.vector.tensor_sub(rng[:], mx[:], mn[:])
            nc.vector.tensor_scalar_add(rng[:], rng[:], EPS)
            inv = gp.tile([4, 1], f32)
            nc.vector.reciprocal(inv[:], rng[:])
            nc.vector.tensor_scalar_mul(inv[:], inv[:], float(IMAGE))
            gn = gp.tile([4, 128], f32)
            nc.vector.tensor_scalar_sub(gn[:], g2[:], mn[:])
            nc.vector.tensor_scalar_mul(gn[:], gn[:], inv[:])

            # transpose [4,128] -> [128,4] on PE (prologue, before warmups)
            tp_ps = mm2ps.tile([128, 4], f32, tag='B')
            nc.tensor.transpose(tp_ps[:], gn[:], idt[:])
            tp = gp.tile([128, 4], f32)
            nc.scalar.copy(tp[:], tp_ps[:])

            # bias_x[r] = -(gxn[b(r), nu(r)] + CX), via batch-select mask
            CX = PW / 2 - 0.5 + PAD      # 31.5
            CY = PW / 2 - 0.4 + PAD      # 31.6
            bias_x = gp.tile([128, 1], f32)
            bias_y = gp.tile([128, 1], f32)
            d01 = gp.tile([128, 1], f32)
            xv = gp.tile([128, 1], f32)
            nc.vector.tensor_sub(d01[:], tp[:, 1:2], tp[:, 0:1])
            nc.vector.scalar_tensor_tensor(xv[:], d01[:], bsel[:], tp[:, 0:1],
                                           ALU.mult, ALU.add)
            nc.vector.tensor_scalar(bias_x[:], xv[:], CX, -1.0,
                                    ALU.add, ALU.mult)
            d23 = gp.tile([128, 1], f32)
            yv = gp.tile([128, 1], f32)
            nc.vector.tensor_sub(d23[:], tp[:, 3:4], tp[:, 2:3])
            nc.vector.scalar_tensor_tensor(yv[:], d23[:], bsel[:], tp[:, 2:3],
                                           ALU.mult, ALU.add)
            nc.vector.tensor_scalar(bias_y[:], yv[:], CY, -1.0,
                                    ALU.add, ALU.mult)

            # ---- E rows: exp(-(t + bias)^2 / SIGMA2) ----
            it = gp.tile([128, ET], f32)
            nc.gpsimd.iota(it[:], pattern=[[1, ET]], base=0,
                           channel_multiplier=0,
                           allow_small_or_imprecise_dtypes=True)
            sqx = gp.tile([128, ET], f32)
            sqy = gp.tile([128, ET], f32)
            E_x = gp.tile([128, ET], f32)
            E_y = gp.tile([128, ET], f32)
            nc.scalar.activation(sqx[:], it[:], AF.Square, bias=bias_x[:],
                                 scale=1.0)
            nc.scalar.activation(E_x[:], sqx[:], AF.Exp, bias=0.0,
                                 scale=-1.0 / SIGMA2)
            nc.scalar.activation(sqy[:], it[:], AF.Square, bias=bias_y[:],
                                 scale=1.0)
            nc.scalar.activation(E_y[:], sqy[:], AF.Exp, bias=0.0,
                                 scale=-1.0 / SIGMA2)
            nc.sync.dma_start(E_x_dram[:], E_x[:])
            nc.sync.dma_start(E_y_dram[:], E_y[:])

            # ---- warmup matmuls (keep PE busy through the prologue) ----
            for w in range(4):
                wps = mm1ps.tile([128, IMAGE], f32, name='wps', tag='p1_0')
                nc.tensor.matmul(wps[:], sqx[:, 0:128], sqx[:, 0:IMAGE],
                                 start=True, stop=True)

            # ---- normalizers: D[r,q'] = T - prefix - suffix; IV = 1/D ----
            def normalizer(E, name):
                T = gp.tile([128, 1], f32, name=f'T{name}')
                nc.vector.reduce_sum(T[:], E[:], axis=AX.X)
                P1 = gp.tile([128, PW], f32, name=f'P1{name}')
                nc.vector.tensor_tensor_scan(P1[:], E[:, 0:PW], E[:, 0:PW],
                                             0.0, ALU.add, ALU.bypass)
                P2 = gp.tile([128, PW - 1], f32, name=f'P2{name}')
                nc.vector.tensor_tensor_scan(P2[:], E[:, WLEN:ET],
                                             E[:, WLEN:ET],
                                             0.0, ALU.add, ALU.bypass)
                T2 = gp.tile([128, 1], f32, name=f'T2{name}')
                nc.vector.scalar_tensor_tensor(T2[:], T[:], EPS,
                                               P2[:, PW - 2:PW - 1],
                                               ALU.add, ALU.subtract)
                Q = gp.tile([128, PW - 1], f32, name=f'Q{name}')
                nc.vector.tensor_sub(Q[:], P2[:], P1[:, 0:PW - 1])
                D = gp.tile([128, PW], f32, name=f'D{name}')
                nc.vector.tensor_copy(D[:, 0:1], T2[:])
                nc.vector.tensor_scalar_add(D[:, 1:PW], Q[:], T2[:])
                REC = gp.tile([128, PW], f32, name=f'REC{name}')
                nc.vector.reciprocal(REC[:], D[:])
                IV = gp.tile([128, PW], f32, name=f'IV{name}')
                nc.vector.transpose(IV[:], REC[:])
                return IV

            IVX = normalizer(E_x, 'x')
            IVY = normalizer(E_y, 'y')

            # ---- main loops ----
            fy_tiles = {}
            t_tiles = {}
            for b in range(BLOC):
                for g in range(NG):
                    m = 2 * g + b
                    fx = fxp.tile([128, IMAGE], f32, name='fx', tag='fx')
                    nc.sync.dma_start(
                        fx[:], AP(E_x_dram, m * ET + PAD,
                                  [[32 * ET, 4], [1, PW], [1, IMAGE]]))
                    fy = fyp.tile([128, IMAGE], f32, name=f'fy{g}',
                                  tag=f'fy{g}')
                    nc.gpsimd.dma_start(
                        fy[:], AP(E_y_dram, m * ET + PAD,
                                  [[32 * ET, 4], [1, PW], [1, IMAGE]]))
                    fy_tiles[g] = fy
                    fxn = fxp.tile([128, IMAGE], f32r, name='fxn', tag='fxn')
                    nc.vector.tensor_scalar_mul(fxn[:], fx[:],
                                                IVX[:, m:m + 1])
                    tg = tgp.tile([128, C * IMAGE], f32r, name=f't{g}',
                                  tag=f't{g}')
                    t_tiles[g] = tg
                    for c in range(C):
                        p1 = mm1ps.tile([128, IMAGE], f32, name=f'p1_{c}',
                                        tag=f'p1_{c}')
                        nc.tensor.matmul(
                            p1[:],
                            psall[b][:, 384 * g + 128 * c:
                                     384 * g + 128 * c + 128],
                            fxn[:], start=True, stop=True)
                        dst = tg[:, IMAGE * c:IMAGE * (c + 1)]
                        if c == 1:
                            nc.vector.tensor_scalar_mul(dst, p1[:],
                                                        IVY[:, m:m + 1])
                        else:
                            nc.scalar.mul(dst, p1[:], IVY[:, m:m + 1])

                for yt in range(2):
                    acc01 = mm2ps.tile([128, 2 * IMAGE], f32, name='acc01',
                                       tag='A')
                    acc2 = mm2ps.tile([128, IMAGE], f32, name='acc2', tag='B')
                    for g in range(NG):
                        lhsT = fy_tiles[g].bitcast(f32r)[:, 128 * yt:
                                                         128 * yt + 128]
                        nc.tensor.matmul(acc01[:], lhsT,
                                         t_tiles[g][:, 0:2 * IMAGE],
                                         start=(g == 0), stop=(g == NG - 1))
                        nc.tensor.matmul(acc2[:], lhsT,
                                         t_tiles[g][:, 2 * IMAGE:3 * IMAGE],
                                         start=(g == 0), stop=(g == NG - 1))
                    for c in range(C):
                        ob = obp.tile([128, IMAGE], f32, name=f'ob{c}',
                                      tag=f'ob{c}')
                        src = acc01[:, IMAGE * c:IMAGE * (c + 1)] if c < 2 \
                            else acc2[:]
                        nc.scalar.mul(ob[:], src, 1.0 / N)
                        nc.scalar.dma_start(
                            y_out[b, c, 128 * yt:128 * yt + 128, :], ob[:])

    _split_multi_waits(nc)
    _PROGRAM = nc
    return nc


def _make_in_maps(brushes: np.ndarray, patches: np.ndarray):
    brushes = np.asarray(brushes, dtype=np.float32)
    patches = np.asarray(patches, dtype=np.float32)
    id4 = np.eye(4, dtype=np.float32)
    r = np.arange(128)
    nu = 4 * ((r % 32) // 2) + (r // 32)
    bsel = (r % 2).astype(np.float32).reshape(128, 1)
    in_maps = []
    for k in range(NCORES):
        bsl = brushes[BLOC * k: BLOC * (k + 1)]        # [2, 64, 2]
        g2 = np.ascontiguousarray(np.stack([
            bsl[0, nu, 0], bsl[1, nu, 0], bsl[0, nu, 1], bsl[1, nu, 1]]))
        psl = patches[BLOC * k: BLOC * (k + 1)]         # [2, 64, 3, 32, 32]
        pr = psl.reshape(BLOC, NG, 4, C, PH, PW)[..., ::-1, ::-1]
        prt = pr.transpose(0, 2, 5, 1, 3, 4)            # [b, j, q', g, c, p']
        pt2 = np.zeros((BLOC, 4, PW, NG, C, 4, PH), dtype=np.float32)
        for j in range(4):
            pt2[:, j, :, :, :, j, :] = prt[:, j]
        pt2 = pt2.reshape(BLOC, 128, NG * C * 128)
        in_maps.append({'g2': g2, 'bsel': bsel, 'pt2': pt2, 'id4': id4})
    return in_maps


def kernel(brushes: np.ndarray, patches: np.ndarray) -> np.ndarray:
    from concourse.bass_utils import run_bass_kernel_spmd

    nc = _build_program()
    in_maps = _make_in_maps(brushes, patches)
    res = run_bass_kernel_spmd(nc, in_maps, list(range(NCORES)))
    out = np.concatenate([res.results[k]['y_out'] for k in range(NCORES)],
                         axis=0)
    return out


# revision 13
# speedup vs baseline: 2.6521x; 1.6135x over previous
"""BrushStroke splat kernel for 8 trn2 NeuronCores (v3).

out[b,c,y,x] = mean_n sum_{p,q} Fy[b,n,y,p] Fx[b,n,x,q] patches[b,n,c,p,q]
with Fx/Fy separable Gaussian filter banks (sigma=0.1) normalized over a
padded spatial axis.

Per core (2 batches of 64 strokes, batch-parallel across cores):
 - E rows E[r,t] = exp(-(t - c_r)^2 / (2 sigma^2)), t in [0,319), one per
   (stroke, batch) on partition r = 32j + 2g + b, stored bf16 as one
   [128, 638] x||y tile and bounced to DRAM.
 - One-time prologue computes all 64x32 window-sum normalizers per side
   (window = T - prefix - suffix via tensor_tensor_scan), reciprocal,
   then a DVE 32x32 block transpose -> per-partition scales IVX/IVY.
 - Per group one bf16 DMA gather [128,512] provides both Fx and Fy
   shifted windows; Fx normalizer is one tensor_scalar_mul; Fy windows
   feed MM2 raw (its normalizer is folded into the MM1 PSUM drain).
 - MM1 per (g,c): full-array bf16 matmul with block-diagonal lhsT
   (zeros embedded host-side, bf16, DMA'd in column chunks).
 - MM2 per (b,g): 2 LDW x 4 chained matmuls into 4 PSUM banks
   (yt0/yt1 x {512-wide c0c1, 256-wide c2}).
 - f32 warmup matmuls during the prologue keep the PE HAM warm.
"""
import sys, types
import numpy as np

IMAGE = 256
PAD = 16
EPS = 1e-7
SIGMA2 = 2.0 * 0.1 ** 2
B, N, C, PH, PW = 16, 64, 3, 32, 32
NCORES = 8
BLOC = B // NCORES          # 2 batches per core
NG = N // 4                 # 16 groups of 4 strokes
ET = IMAGE + 2 * PAD + PW - 1   # 319: E row length
WLEN = IMAGE + 2 * PAD          # 288: padded-axis (normalizer window) length


def _install_patches():
    if 'antenv.axon_hooks' not in sys.modules:
        mod = types.ModuleType('antenv.axon_hooks')
        mod._hook = None
        mod.set_axon_ntff_profile_hook = lambda h: setattr(mod, '_hook', h)
        mod.get_axon_ntff_profile_hook = lambda: mod._hook
        sys.modules['antenv.axon_hooks'] = mod
        try:
            from trn_agent_boot.trn_boot import _ntff_profile_via_ctypes
            hook = _ntff_profile_via_ctypes('/opt/axon/libaxon_pjrt.so')
            if hook is not None:
                mod.set_axon_ntff_profile_hook(hook)
        except Exception:
            pass

    import concourse.tile as tile
    import concourse.bass_utils as bass_utils
    from concourse.vector_clock import ScopedClock

    bass_utils.upload_artifacts = lambda tmpdir: 'local://' + tmpdir

    if getattr(tile.TileContext._drain_and_barrier, '_patched', False):
        return

    def _drain_and_barrier(self, tick_clock, wait_clock):
        nc = self.nc
        drain_inst = nc.sync.drain()
        wait_clock.add_sem_waits(
            drain_inst.ins, ScopedClock({None: tick_clock.global_clock}))
        si = drain_inst.ins.sync_info
        waits = list(si.on_wait or [])
        si.on_wait = []
        for w in waits:
            nop = nc.sync.nop()
            nop.ins.sync_info = type(si)(on_wait=[w], on_update=[])
        nc.all_engine_barrier()
        popped = nc._tile_sem_poison_stack.pop()
        assert popped is self._sem_poison
        nc.clear_and_free_semaphores(list(self.sems.allocated().values()))
        nc.all_engine_barrier()

    _drain_and_barrier._patched = True
    tile.TileContext._drain_and_barrier = _drain_and_barrier


def _split_multi_waits(nc):
    """This walrus accepts at most one sync wait per instruction; hoist
    extras onto same-engine NoOps inserted just before."""
    import bass_rust
    n_new = [0]

    def fresh_nop(engine, wait, si_type):
        n_new[0] += 1
        nop = bass_rust.InstNoOp(name=f'I-waitsplit-{n_new[0]}', ins=[], outs=[])
        nop.engine = engine
        nop.sync_info = si_type(on_wait=[wait], on_update=[])
        return nop

    for fn in nc.m.functions:
        for blk in fn.blocks:
            insts = blk.instructions
            i = 0
            while i < len(insts):
                inst = insts[i]
                si = inst.sync_info
                if si is not None and si.on_wait and len(si.on_wait) > 1:
                    waits = list(si.on_wait)
                    si.on_wait = [waits[-1]]
                    for k, w in enumerate(waits[:-1]):
                        insts.insert(i + k, fresh_nop(inst.engine, w, type(si)))
                    i += len(waits) - 1
                i += 1


_PROGRAM = None


def _build_program():
    global _PROGRAM
    if _PROGRAM is not None:
        return _PROGRAM
    _install_patches()
    import concourse.bass as bass
    import concourse.tile as tile
    from concourse import mybir
    from bass_rust import AP

    f32 = mybir.dt.float32
    bf16 = mybir.dt.bfloat16
    AF = mybir.ActivationFunctionType
    AX = mybir.AxisListType
    ALU = mybir.AluOpType

    nc = bass.Bass('TRN2', target_bir_lowering=False, debug=False,
                   num_devices=NCORES)
    # per-core inputs:
    #  g2w  [5,133]: cols 0:128 rows 0-3 brush coords (x_b0,x_b1,y_b0,y_b1;
    #        col r = stroke nu(r)), row 4 = batch-select r%2;
    #        cols 128:133 = 5x5 identity
    #  pt2  [2,128,6144] bf16: full block-diagonal lhsT (zeros embedded)
    g2w_in = nc.declare_dram_parameter('g2w', [5, 133], f32, isOutput=False)
    pt_in = nc.declare_dram_parameter('pt2', [BLOC, 128, 128 * C * NG], bf16,
                                      isOutput=False)
    y_out = nc.declare_dram_parameter('y_out', [BLOC, C, IMAGE, IMAGE], f32,
                                      isOutput=True)

    E_dram = nc.dram_tensor('E_dram', [128, 2 * ET], bf16)

    with tile.TileContext(nc) as tc:
        with tc.tile_pool(name='glob', bufs=1) as gp, \
             tc.tile_pool(name='fxyp', bufs=2) as fxyp, \
             tc.tile_pool(name='fxnp', bufs=4) as fxnp, \
             tc.tile_pool(name='tgp', bufs=1) as tgp, \
             tc.tile_pool(name='obp', bufs=2) as obp, \
             tc.tile_pool(name='mm1ps', bufs=2, space='PSUM') as mm1ps, \
             tc.tile_pool(name='mm2ps', bufs=1, space='PSUM') as mm2ps:
            # ---- input DMAs ----
            g2w = gp.tile([5, 133], f32)
            nc.sync.dma_start(g2w[:], g2w_in[:])
            g2 = g2w[0:4, 0:128]
            idt = g2w[:, 128:133]

            psall = []
            for b in range(BLOC):
                ps = gp.tile([128, 128 * C * NG], bf16, name=f'psall{b}')
                psall.append(ps)
            CHUNK = 1536
            for ch in range(4):          # batch-0 fills, early, on sync
                nc.sync.dma_start(psall[0][:, CHUNK * ch:CHUNK * (ch + 1)],
                                  pt_in[0, :, CHUNK * ch:CHUNK * (ch + 1)])

            # ---- iotas (gpsimd) ----
            it = gp.tile([128, ET], f32)
            nc.gpsimd.iota(it[:], pattern=[[1, ET]], base=0,
                           channel_multiplier=0,
                           allow_small_or_imprecise_dtypes=True)
            # t^2 early (also the warmup matmul operand)
            t2 = gp.tile([128, ET], f32)
            nc.vector.tensor_mul(t2[:], it[:], it[:])

            # ---- brush normalization -> bias vectors ----
            g25 = g2w[0:5, 0:128]
            mn = gp.tile([5, 1], f32)
            mx = gp.tile([5, 1], f32)
            nc.vector.tensor_reduce(mn[:], g25, axis=AX.X, op=ALU.min)
            nc.vector.reduce_max(mx[:], g25, axis=AX.X)
            rng = gp.tile([5, 1], f32)
            nc.vector.tensor_scalar(rng[:], mx[:], mn[:], EPS,
                                    ALU.subtract, ALU.add)
            inv = gp.tile([5, 1], f32)
            nc.vector.reciprocal(inv[:], rng[:])
            gn = gp.tile([5, 128], f32)
            nc.vector.tensor_scalar(gn[:], g25, mn[:], inv[:],
                                    ALU.subtract, ALU.mult)

            tp_ps = mm2ps.tile([128, 5], f32, tag='A0')
            nc.tensor.transpose(tp_ps[:], gn[:], idt)
            tp = gp.tile([128, 5], f32)
            nc.scalar.copy(tp[:], tp_ps[:])
            bs = tp[:, 4:5]

            # bias_x[r] = -(256*gxn[b(r),nu(r)] + CX)
            CXC = PW / 2 - 0.5 + PAD      # 31.5
            CYC = PW / 2 - 0.4 + PAD      # 31.6
            bias = {}
            for nmo, (c0, c1, CC) in {'x': (0, 1, CXC),
                                      'y': (2, 3, CYC)}.items():
                d01 = gp.tile([128, 1], f32, name=f'd{nmo}')
                v = gp.tile([128, 1], f32, name=f'v{nmo}')
                bi = gp.tile([128, 1], f32, name=f'bias{nmo}')
                nc.vector.tensor_sub(d01[:], tp[:, c1:c1 + 1], tp[:, c0:c0 + 1])
                nc.vector.scalar_tensor_tensor(v[:], d01[:], bs,
                                               tp[:, c0:c0 + 1],
                                               ALU.mult, ALU.add)
                nc.vector.tensor_scalar(bi[:], v[:], -float(IMAGE), CC,
                                        ALU.mult, ALU.subtract)
                bias[nmo] = bi

            # ---- E rows: exp(-(t+b)^2/S2); (t+b) first to avoid fp32
            # cancellation in the expanded square
            E = gp.tile([128, 2 * ET], bf16)
            for nmo, off in (('x', 0), ('y', ET)):
                dd = gp.tile([128, ET], f32, name=f'dd{nmo}')
                nc.vector.tensor_scalar_add(dd[:], it[:], bias[nmo][:])
                sq = gp.tile([128, ET], f32, name=f'sq{nmo}')
                nc.vector.tensor_mul(sq[:], dd[:], dd[:])
                nc.scalar.activation(E[:, off:off + ET], sq[:], AF.Exp,
                                     bias=0.0, scale=-1.0 / SIGMA2)
            nc.sync.dma_start(E_dram[:], E[:])

            # ---- warmup matmuls (keep PE busy through the prologue) ----
            for w in range(4):
                wps = mm1ps.tile([128, 512], f32, name='wps', tag='p01')
                nc.tensor.matmul(wps[:, 0:IMAGE], t2[:, 0:128], t2[:, 0:IMAGE],
                                 start=True, stop=True)

            # ---- normalizers: D[r,q'] = T - prefix - suffix; IV = 1/D ----
            def normalizer(off, name):
                Es = E[:, off:off + ET]
                T = gp.tile([128, 1], f32, name=f'T{name}')
                nc.vector.reduce_sum(T[:], Es, axis=AX.X)
                P1 = gp.tile([128, PW], f32, name=f'P1{name}')
                nc.vector.tensor_tensor_scan(P1[:], Es[:, 0:PW], Es[:, 0:PW],
                                             0.0, ALU.add, ALU.bypass)
                P2 = gp.tile([128, PW - 1], f32, name=f'P2{name}')
                nc.vector.tensor_tensor_scan(P2[:], Es[:, WLEN:ET],
                                             Es[:, WLEN:ET],
                                             0.0, ALU.add, ALU.bypass)
                T2 = gp.tile([128, 1], f32, name=f'T2{name}')
                nc.vector.scalar_tensor_tensor(T2[:], T[:], EPS,
                                               P2[:, PW - 2:PW - 1],
                                               ALU.add, ALU.subtract)
                Q = gp.tile([128, PW - 1], f32, name=f'Q{name}')
                nc.vector.tensor_sub(Q[:], P2[:], P1[:, 0:PW - 1])
                D = gp.tile([128, PW], f32, name=f'D{name}')
                nc.vector.tensor_copy(D[:, 0:1], T2[:])
                nc.vector.tensor_scalar_add(D[:, 1:PW], Q[:], T2[:])
                REC = gp.tile([128, PW], f32, name=f'REC{name}')
                nc.vector.reciprocal(REC[:], D[:])
                IV = gp.tile([128, PW], f32, name=f'IV{name}')
                nc.vector.transpose(IV[:], REC[:])
                return IV

            IVX = normalizer(0, 'x')
            IVY = normalizer(ET, 'y')

            # batch-1 lhsT fills (scalar queue, after the E activations)
            for ch in range(4):
                nc.scalar.dma_start(psall[1][:, CHUNK * ch:CHUNK * (ch + 1)],
                                    pt_in[1, :, CHUNK * ch:CHUNK * (ch + 1)])

            # ---- main loops ----
            for b in range(BLOC):
                fxy_tiles = {}
                tg_tiles = {}
                for g in range(NG):
                    m = 2 * g + b
                    fxg = fxnp.tile([128, IMAGE], bf16, name='fxg', tag='fxg')
                    nc.sync.dma_start(
                        fxg[:], AP(E_dram, m * 2 * ET + PAD,
                                   [[2 * ET * 32, 4], [1, PW], [1, IMAGE]]))
                    fyg = fxyp.tile([128, IMAGE], bf16, name=f'fy{g}',
                                    tag=f'fy{g}')
                    nc.gpsimd.dma_start(
                        fyg[:], AP(E_dram, m * 2 * ET + ET + PAD,
                                   [[2 * ET * 32, 4], [1, PW], [1, IMAGE]]))
                    fxy_tiles[g] = fyg
                    fxn = fxnp.tile([128, IMAGE], bf16, name='fxn', tag='fxn')
                    nc.vector.tensor_scalar_mul(fxn[:], fxg[:],
                                                IVX[:, m:m + 1])
                    tg = tgp.tile([128, C * IMAGE], bf16, name=f't{g}',
                                  tag=f't{g}')
                    tg_tiles[g] = tg
                    p01 = mm1ps.tile([128, 512], f32, name='p01', tag='p01')
                    p2 = mm1ps.tile([128, IMAGE], f32, name='p2', tag='p2')
                    for c in range(C):
                        dst = p01[:, IMAGE * c:IMAGE * (c + 1)] if c < 2 \
                            else p2[:]
                        nc.tensor.matmul(
                            dst,
                            psall[b][:, 384 * g + 128 * c:
                                     384 * g + 128 * c + 128],
                            fxn[:], start=True, stop=True,
                            skip_group_check=(c == 1))
                    if g % 2 == 0:
                        nc.scalar.mul(tg[:, 0:512], p01[:], IVY[:, m:m + 1])
                        nc.vector.tensor_scalar_mul(tg[:, 512:768], p2[:],
                                                    IVY[:, m:m + 1])
                    else:
                        nc.vector.tensor_scalar_mul(tg[:, 0:512], p01[:],
                                                    IVY[:, m:m + 1])
                        nc.scalar.mul(tg[:, 512:768], p2[:], IVY[:, m:m + 1])

                accs = {}
                for yt in range(2):
                    accs[(yt, 'A')] = mm2ps.tile([128, 512], f32,
                                                 name=f'A{yt}', tag=f'A{yt}')
                    accs[(yt, 'B')] = mm2ps.tile([128, IMAGE], f32,
                                                 name=f'B{yt}', tag=f'B{yt}')
                for g in range(NG):
                    st, sp = (g == 0), (g == NG - 1)
                    tg = tg_tiles[g]
                    for yt in range(2):
                        l = fxy_tiles[g][:, 128 * yt:128 * yt + 128]
                        nc.tensor.matmul(accs[(yt, 'A')][:], l, tg[:, 0:512],
                                         start=st, stop=sp)
                        nc.tensor.matmul(accs[(yt, 'B')][:], l, tg[:, 512:768],
                                         start=st, stop=sp)
                for yt in range(2):
                    for c in range(C):
                        ob = obp.tile([128, IMAGE], f32, name=f'ob{yt}{c}',
                                      tag=f'ob{yt}{c}')
                        src = accs[(yt, 'A')][:, IMAGE * c:IMAGE * (c + 1)] \
                            if c < 2 else accs[(yt, 'B')][:]
                        nc.scalar.mul(ob[:], src, 1.0 / N)
                        deng = nc.scalar if c % 2 == 0 else nc.gpsimd
                        deng.dma_start(
                            y_out[b, c, 128 * yt:128 * yt + 128, :], ob[:])

    _split_multi_waits(nc)
    _PROGRAM = nc
    return nc


def _make_in_maps(brushes: np.ndarray, patches: np.ndarray):
    import ml_dtypes
    brushes = np.asarray(brushes, dtype=np.float32)
    patches = np.asarray(patches, dtype=np.float32)
    r = np.arange(128)
    nu = 4 * ((r % 32) // 2) + (r // 32)
    in_maps = []
    for k in range(NCORES):
        bsl = brushes[BLOC * k: BLOC * (k + 1)]        # [2, 64, 2]
        g2w = np.zeros((5, 133), dtype=np.float32)
        g2w[0:4, 0:128] = np.stack([
            bsl[0, nu, 0], bsl[1, nu, 0], bsl[0, nu, 1], bsl[1, nu, 1]])
        g2w[4, 0:128] = (r % 2).astype(np.float32)
        g2w[:, 128:133] = np.eye(5, dtype=np.float32)
        psl = patches[BLOC * k: BLOC * (k + 1)]         # [2, 64, 3, 32, 32]
        pr = psl.reshape(BLOC, NG, 4, C, PH, PW)[..., ::-1, ::-1]
        prt = pr.transpose(0, 2, 5, 1, 3, 4)            # [b, j, q', g, c, p']
        pt2 = np.zeros((BLOC, 4, PW, NG, C, 4, PH), dtype=np.float32)
        for j in range(4):
            pt2[:, j, :, :, :, j, :] = prt[:, j]
        pt2 = pt2.reshape(BLOC, 128, NG * C * 128).astype(ml_dtypes.bfloat16)
        in_maps.append({'g2w': g2w, 'pt2': pt2})
    return in_maps


def kernel(brushes: np.ndarray, patches: np.ndarray) -> np.ndarray:
    from concourse.bass_utils import run_bass_kernel_spmd

    nc = _build_program()
    in_maps = _make_in_maps(brushes, patches)
    res = run_bass_kernel_spmd(nc, in_maps, list(range(NCORES)))
    out = np.concatenate([res.results[k]['y_out'] for k in range(NCORES)],
                         axis=0)
    return out
